# revision 2
# baseline (speedup 1.0000x reference)
"""Trainium2 Bass kernel for the MultiHeadAttention transformer block, v3.

Sharding: 8 cores, core c handles batch b=c//2 and query-row half
(c%2)*1024 .. +1024, all 8 heads.  Fully independent cores.

v2 on top of the transposed-layout baseline:
  - per-head mask compaction: the key mask is ~50% zeros, so each
    (head, batch) attends to ~1024 of 2048 keys.  The host gathers the
    active keys per head; on-chip K/V projections, QK, exp and AV run
    over NT=ceil(max_active/128) tiles (9 for a balanced mask) instead
    of 16 — an exact transformation (masked keys contribute exp(-inf)=0
    in the reference too).  K projection turns per-head (moving = the
    head's own gathered keys); pad slots keep the -1e9 bias.
  - the V projection runs in fp8-e4m3 DoubleRow (inputs and Wv
    quantized on the host), halving its PE time; its f32 psum output
    is stored to bf16 V_s, so only the input quantization (~1.2e-2
    end-to-end) is added.  Everything else stays bf16: e4m3 on the
    QK path or on softmax weights measurably exceeds the error budget
    (concentrated softmax does not average quantization noise).
  - everything still lives transposed on chip [channel on partitions,
    sequence on free], so no on-chip transposes anywhere: scores are
    S^T with the key mask as a per-partition Exp bias, A@V keeps V
    stationary with an appended ones-column for the softmax row sums,
    LayerNorm stats come from ones-column matmuls, and the fc output
    projection produces out^T which the host un-transposes for free.
"""

import sys

if "/opt/trn_rl_repo" not in sys.path:
    sys.path.insert(0, "/opt/trn_rl_repo")

import numpy as np

import concourse.bacc as bacc
import concourse.bass as bass
import concourse.tile as tile
from concourse import mybir
from concourse.bass_utils import run_bass_kernel_spmd

H, D, DK, DV = 8, 512, 64, 64
B, L = 4, 2048
P = 128
LQ = L // 2          # query rows per core
NCORES = 8
EPS = 1e-5
NEG = -1e9 / 8.0     # masked score after the /temperature divide
F32 = mybir.dt.float32
BF16 = mybir.dt.bfloat16
FP8 = mybir.dt.float8e4
AF = mybir.ActivationFunctionType
Alu = mybir.AluOpType
DR = mybir.MatmulPerfMode.DoubleRow

DT = D // P     # 4 d-tiles
NB = LQ // 512  # 2 psum-bank columns of queries

_CACHE = {}


def _bcast(ap, parts):
    """Partition-broadcast view of a [1, n] DRAM AP for DMA replication."""
    return ap.to_broadcast([parts] + list(ap.shape[1:]))


def _emit(nc, tc, NT):
    NPAD = NT * P
    qT = nc.dram_tensor("qT", [P, DT, LQ], BF16, kind="ExternalInput")
    kT = nc.dram_tensor("kT", [H, P, DT, NPAD], BF16, kind="ExternalInput")
    vT8 = nc.dram_tensor("vT8", [H, P, DT, NPAD], FP8, kind="ExternalInput")
    qresT = nc.dram_tensor("qresT", [P, DT, LQ], BF16, kind="ExternalInput")
    WqT = nc.dram_tensor("WqT", [P, DT, D], BF16, kind="ExternalInput")
    WkT = nc.dram_tensor("WkT", [P, DT, D], BF16, kind="ExternalInput")
    WvT8 = nc.dram_tensor("WvT8", [P, DT, D], FP8, kind="ExternalInput")
    fcwT = nc.dram_tensor("fcwT", [P, DT, D], BF16, kind="ExternalInput")
    mb = nc.dram_tensor("mb", [P, H * NT], F32, kind="ExternalInput")
    vecs = nc.dram_tensor("vecs", [5, P, DT], F32, kind="ExternalInput")
    out = nc.dram_tensor("out", [P, DT, LQ], BF16, kind="ExternalOutput")

    with (
        tc.tile_pool(name="consts", bufs=1) as consts,
        tc.tile_pool(name="projout", bufs=1) as projout,
        tc.tile_pool(name="dramp", bufs=3, space="DRAM") as dramp,
        tc.tile_pool(name="psA", bufs=2, space="PSUM") as psProj,
    ):
        # ---- constants resident for the whole kernel ----
        mb_s = consts.tile([P, H * NT], F32)
        gbT = consts.tile([P, 5, DT], F32)   # g0,b0,g1,b1,fcb as [p, dt]
        eps_t = consts.tile([P, 1], F32)
        nc.vector.memset(eps_t, EPS)
        ones_st = consts.tile([P, 1], BF16)  # stats reduction column
        nc.vector.memset(ones_st, 1.0)
        ones_r1 = consts.tile([1, P], BF16)  # rank-1 broadcast row
        nc.vector.memset(ones_r1, 1.0)
        warm = consts.tile([P, 512], BF16)   # PE warm-up fodder
        nc.vector.memset(warm[:, :], 0.0)
        expw = consts.tile([1, 1], F32)      # preload the Exp table set
        nc.scalar.activation(out=expw, in_=eps_t[0:1, 0:1], func=AF.Exp,
                             bias=eps_t[0:1, :], scale=1.0)

        # ---- persistent big tiles ----
        QT_s = projout.tile([P, DT, LQ], BF16)         # Q^T  [ch, lq]
        qresT_s = projout.tile([P, DT, LQ], BF16)

        with tc.tile_pool(name="statin", bufs=1) as statin:
            # x and x^2 interleaved so LN stats matmuls sum both at once
            xx = statin.tile([P, DT, 2, LQ], BF16, tag="xx")
            xbf = xx[:, :, 0, :]
            x2bf = xx[:, :, 1, :]

            # ====== phases A+B fused: per-head projections + attention =====
            with (
                tc.tile_pool(name="inp", bufs=1) as inp,
                tc.tile_pool(name="wts", bufs=1) as wts,
                tc.tile_pool(name="kvin", bufs=2) as kvin,
                tc.tile_pool(name="ktzp", bufs=1) as ktzp,
                tc.tile_pool(name="vsp", bufs=1) as vsp,
                tc.tile_pool(name="pT", bufs=10) as pTp,
                tc.tile_pool(name="bcsp", bufs=4) as bcsp,
            ):
                for w in range(20):
                    wps = psProj.tile([P, 512], F32, tag="proj",
                                      name=f"warm{w}")
                    nc.tensor.matmul(wps[:, :], warm[:, 0:P], warm[:, :],
                                     start=True, stop=True)
                # persistent KTz / V_s buffers; their constant regions (the
                # dead 64 rows, the softmax-denominator ones column) are
                # initialized once here, under the initial DMA wait
                ktzs = [ktzp.tile([P, NPAD], BF16, tag=f"ktzb{i}",
                                  name=f"ktzb{i}") for i in range(4)]
                for i in range(4):
                    par = i % 2
                    nc.vector.memset(
                        ktzs[i][(1 - par) * 64:(2 - par) * 64, :], 0.0)
                vss = [vsp.tile([P, NT, DV + 1], BF16, tag=f"vsb{i}",
                                name=f"vsb{i}") for i in range(2)]
                for i in range(2):
                    nc.vector.memset(vss[i][:, :, DV:DV + 1], 1.0)
                WqT_s = wts.tile([P, DT, D], BF16)
                WkT_s = wts.tile([P, DT, D], BF16)
                Wv8_s = wts.tile([P, DT, D], FP8)
                qT_s = inp.tile([P, DT, LQ], BF16)

                def load_weights_and_consts():
                    # emission order = DMA service order: the head-0 critical
                    # path (Wk, then kT0/vT0 via load_kv) goes first
                    for dt in range(DT):
                        nc.sync.dma_start(out=WkT_s[:, dt, :],
                                          in_=WkT[:, dt, :])

                def load_rest():
                    nc.sync.dma_start(out=mb_s, in_=mb[:, :])
                    for i in range(5):
                        nc.sync.dma_start(out=gbT[:, i, :], in_=vecs[i, :, :])
                    for dt in range(DT):
                        nc.sync.dma_start(out=WqT_s[:, dt, :],
                                          in_=WqT[:, dt, :])
                    for dt in range(DT):
                        nc.sync.dma_start(out=qT_s[:, dt, :], in_=qT[:, dt, :])
                    nc.sync.dma_start(out=Wv8_s, in_=WvT8[:, :, :])
                    for dt in range(DT):
                        nc.sync.dma_start(out=qresT_s[:, dt, :],
                                          in_=qresT[:, dt, :])

                def load_kv(h):
                    kT_s = kvin.tile([P, DT, NPAD], BF16, tag="kT",
                                     name=f"kT{h}")
                    vT8_s = kvin.tile([P, DT, NPAD], FP8, tag="vT",
                                      name=f"vT{h}")
                    for dt in range(DT):
                        nc.sync.dma_start(out=kT_s[:, dt, :],
                                          in_=kT[h, :, dt, :])
                    nc.sync.dma_start(out=vT8_s, in_=vT8[h, :, :, :])
                    return kT_s, vT8_s

                def emit_qproj(m):
                    pss = [psProj.tile([P, 512], F32, tag="proj",
                                       name=f"psq{m}_{j}") for j in range(NB)]
                    for dt in range(DT):
                        for jb in range(NB):
                            nc.tensor.matmul(
                                pss[jb][:, :],
                                WqT_s[:, dt, m * P:(m + 1) * P],
                                qT_s[:, dt, jb * 512:(jb + 1) * 512],
                                start=(dt == 0), stop=(dt == DT - 1))
                    for jb in range(NB):
                        nc.scalar.copy(
                            out=QT_s[:, m, jb * 512:(jb + 1) * 512],
                            in_=pss[jb][:, :])

                def emit_kproj(h, kT_s):
                    """Project head h's gathered keys into its parity rows of
                    a fresh KTz buffer; the other 64 rows are zeroed."""
                    par = h % 2
                    ktz = ktzs[par + 2 * ((h // 2) % 2)]
                    for c0 in range(0, NPAD, 512):
                        cw = min(512, NPAD - c0)
                        ps = psProj.tile([P, 512], F32, tag="proj",
                                         name=f"psk{h}_{c0}")
                        for dt in range(DT):
                            nc.tensor.matmul(
                                ps[0:64, 0:cw],
                                WkT_s[:, dt, h * DK:(h + 1) * DK],
                                kT_s[:, dt, c0:c0 + cw],
                                start=(dt == 0), stop=(dt == DT - 1))
                        nc.scalar.copy(
                            out=ktz[par * 64:(par + 1) * 64, c0:c0 + cw],
                            in_=ps[0:64, 0:cw])
                    return ktz

                def emit_vproj_half(h, t, i, vps, vT8_s):
                    nc.tensor.matmul(
                        vps[:, 0:DV],
                        vT8_s[:, 2 * i:2 * i + 2, t * P:(t + 1) * P],
                        Wv8_s[:, 2 * i:2 * i + 2, h * DV:(h + 1) * DV],
                        start=(i == 0), stop=(i == 1),
                        perf_mode=DR, skip_group_check=True)

                def emit_attn(h, ktz, vT8_s):
                    mt = h // 2
                    V_s = vss[h % 2]
                    avs = [psProj.tile([DV + 1, 512], F32, tag="av", bufs=2,
                                       name=f"av{h}_{j}") for j in range(NB)]
                    vps = psProj.tile([P, 512], F32, tag="proj",
                                      name=f"psv{h}_0")
                    for i in range(2):
                        emit_vproj_half(h, 0, i, vps, vT8_s)
                    nc.vector.tensor_copy(V_s[:, 0, 0:DV], vps[:, 0:DV])
                    for t in range(NT):
                        # vproj for tile t+1, halves interleaved between the
                        # long QK matmuls so its LDWEIGHTS stay hidden
                        if t + 1 < NT:
                            vps = psProj.tile([P, 512], F32, tag="proj",
                                              name=f"psv{h}_{t + 1}")
                            emit_vproj_half(h, t + 1, 0, vps, vT8_s)
                        ps = psProj.tile([P, LQ], F32, tag="qk", bufs=2)
                        nc.tensor.matmul(
                            ps[:, 0:512], ktz[:, t * P:(t + 1) * P],
                            QT_s[:, mt, 0:512], start=True, stop=True)
                        if t + 1 < NT:
                            emit_vproj_half(h, t + 1, 1, vps, vT8_s)
                            nc.vector.tensor_copy(V_s[:, t + 1, 0:DV],
                                                  vps[:, 0:DV])
                        nc.tensor.matmul(
                            ps[:, 512:1024], ktz[:, t * P:(t + 1) * P],
                            QT_s[:, mt, 512:1024], start=True, stop=True)
                        pt = pTp.tile([P, LQ], BF16, tag="pT")
                        nc.scalar.activation(
                            out=pt[:, :], in_=ps[:, :], func=AF.Exp,
                            bias=mb_s[:, h * NT + t:h * NT + t + 1],
                            scale=1.0 / 8.0)
                        for jb in range(NB):
                            nc.tensor.matmul(
                                avs[jb][:, :], V_s[:, t, :],
                                pt[:, jb * 512:(jb + 1) * 512],
                                start=(t == 0), stop=(t == NT - 1))
                    po = (h % 2) * DV
                    for jb in range(NB):
                        sl = slice(jb * 512, (jb + 1) * 512)
                        stg = bcsp.tile([DV + 1, 512], F32, tag="stg")
                        nc.vector.tensor_copy(stg, avs[jb][:, :])
                        rcd = dramp.tile([1, 512], F32, tag="rcd",
                                         name=f"rcd{h}_{jb}")
                        nc.sync.dma_start(out=rcd, in_=stg[DV:DV + 1, :])
                        bcs = bcsp.tile([DV, 512], F32, tag="bcs")
                        nc.gpsimd.dma_start(out=bcs,
                                            in_=_bcast(rcd[0:1, :], DV))
                        nc.vector.reciprocal_approx_fast(out=bcs, in_=bcs)
                        nc.vector.tensor_mul(
                            xbf[po:po + DV, mt, sl], stg[0:DV, :],
                            bcs[:, :])
                    if h % 2 == 1:
                        # channel tile kt complete: residual + stat inputs
                        # (last pair on the fast DVE — it gates phase C)
                        kt = h // 2
                        eng = nc.vector if h == H - 1 else nc.gpsimd
                        eng.tensor_add(xbf[:, kt, :], xbf[:, kt, :],
                                       qresT_s[:, kt, :])
                        eng.tensor_mul(x2bf[:, kt, :], xbf[:, kt, :],
                                       xbf[:, kt, :])

                load_weights_and_consts()
                kv = load_kv(0)
                load_rest()
                for h in range(H):
                    if h + 1 < H:
                        kv_next = load_kv(h + 1)
                    ktz = emit_kproj(h, kv[0])
                    if h % 2 == 0:
                        emit_qproj(h // 2)
                    emit_attn(h, ktz, kv[1])
                    if h + 1 < H:
                        kv = kv_next

            # ============ phase C: LN0 -> fc -> LN1 (all ^T, bf16) ========
            with (
                tc.tile_pool(name="lnp", bufs=1) as lnp,
                tc.tile_pool(name="chain", bufs=2) as chain,
                tc.tile_pool(name="bcB", bufs=2) as bcB,
                tc.tile_pool(name="wfc", bufs=1) as wfc,
            ):
                outT = lnp.tile([P, DT, LQ], BF16)
                yy = lnp.tile([P, DT, 2, LQ], BF16)
                ybf = yy[:, :, 0, :]
                y2bf = yy[:, :, 1, :]

                def ln_stats(xy, nh, label, ring):
                    sl = slice(nh * 512, (nh + 1) * 512)
                    s1 = psProj.tile([1, 512], F32, tag=ring,
                                     name=f"s1{label}{nh}")
                    s2 = psProj.tile([1, 512], F32, tag=ring,
                                     name=f"s2{label}{nh}")
                    for kt in range(DT):
                        nc.tensor.matmul(s1[:, :], ones_st[:, :],
                                         xy[:, kt, 0, sl],
                                         start=(kt == 0), stop=(kt == DT - 1))
                        nc.tensor.matmul(s2[:, :], ones_st[:, :],
                                         xy[:, kt, 1, sl],
                                         start=(kt == 0), stop=(kt == DT - 1))
                    return s1, s2

                def ln_chain(s1, s2, nh, label, ring):
                    mu = chain.tile([1, 512], F32, tag="mu")
                    nc.vector.tensor_scalar_mul(mu, s1[:, :], 1.0 / D)
                    var = chain.tile([1, 512], F32, tag="var")
                    nc.vector.tensor_mul(var, mu, mu)
                    msq = chain.tile([1, 512], F32, tag="msq")
                    nc.vector.tensor_scalar_mul(msq, s2[:, :], 1.0 / D)
                    nc.vector.tensor_sub(var, msq, var)
                    nc.scalar.activation(out=var, in_=var, func=AF.Sqrt,
                                         bias=eps_t[0:1, :])
                    rstd = chain.tile([1, 512], F32, tag="rstd")
                    nc.vector.reciprocal_approx_fast(out=rstd, in_=var)
                    mrb = chain.tile([1, 2, 512], BF16, tag="mrb")
                    nc.vector.tensor_copy(mrb[:, 0, :], mu[0:1, :])
                    nc.vector.tensor_copy(mrb[:, 1, :], rstd[0:1, :])
                    bb0 = psProj.tile([P, 512], F32, tag=ring,
                                      name=f"bb0{label}{nh}")
                    nc.tensor.matmul(bb0[:, :], ones_r1[:, :],
                                     mrb[:, 0, :], start=True, stop=True)
                    bb1 = psProj.tile([P, 512], F32, tag=ring,
                                      name=f"bb1{label}{nh}")
                    nc.tensor.matmul(bb1[:, :], ones_r1[:, :],
                                     mrb[:, 1, :], start=True, stop=True)
                    bc = bcB.tile([P, 2, 512], BF16, tag="bc",
                                  name=f"bc{label}{nh}")
                    nc.vector.tensor_copy(bc[:, 0, :], bb0[:, :])
                    nc.vector.tensor_copy(bc[:, 1, :], bb1[:, :])
                    return bc

                def ln_apply(xy, bc, g_idx, b_idx, nh, final_out=None):
                    sl = slice(nh * 512, (nh + 1) * 512)
                    for kt in range(DT):
                        xb = xy[:, kt, 0, sl]
                        nc.vector.tensor_sub(xb, xb, bc[:, 0, :])
                        nc.vector.tensor_mul(xb, xb, bc[:, 1, :])
                        tgt = xb if final_out is None else final_out[:, kt, sl]
                        nc.scalar.activation(
                            out=tgt, in_=xb, func=AF.Identity,
                            bias=gbT[:, b_idx, kt:kt + 1],
                            scale=gbT[:, g_idx, kt:kt + 1])

                fcwT_s = wfc.tile([P, DT, D], BF16)
                nc.sync.dma_start(out=fcwT_s, in_=fcwT[:, :, :])

                def emit_fc(nh):
                    sl = slice(nh * 512, (nh + 1) * 512)
                    # fc for this half; y = fc + fc_b + LN0 residual (bf16)
                    for m in range(DT):
                        ps = psProj.tile([P, 512], F32, tag="av",
                                         name=f"fc{m}_{nh}")
                        for dt in range(DT):
                            nc.tensor.matmul(
                                ps[:, :],
                                fcwT_s[:, dt, m * P:(m + 1) * P],
                                xx[:, dt, 0, sl],
                                start=(dt == 0), stop=(dt == DT - 1))
                        nc.scalar.activation(
                            out=yy[:, m, 0, sl], in_=ps[:, :],
                            func=AF.Identity, bias=gbT[:, 4, m:m + 1])
                        nc.gpsimd.tensor_add(yy[:, m, 0, sl], yy[:, m, 0, sl],
                                             xx[:, m, 0, sl])
                        nc.vector.tensor_mul(yy[:, m, 1, sl], yy[:, m, 0, sl],
                                             yy[:, m, 0, sl])

                def emit_out(nh):
                    sl = slice(nh * 512, (nh + 1) * 512)
                    for kt in range(DT):
                        nc.sync.dma_start(out=out[:, kt, sl],
                                          in_=outT[:, kt, sl])

                # stats first, chains and applies interleaved across the
                # two query halves so PE matmuls cover the serial LN chains
                sA0 = ln_stats(xx, 0, "a", "qk")
                cA0 = ln_chain(*sA0, 0, "a", "qk")
                sA1 = ln_stats(xx, 1, "a", "proj")
                ln_apply(xx, cA0, 0, 1, 0)
                cA1 = ln_chain(*sA1, 1, "a", "proj")
                emit_fc(0)
                ln_apply(xx, cA1, 0, 1, 1)
                sB0 = ln_stats(yy, 0, "b", "qk")
                cB0 = ln_chain(*sB0, 0, "b", "qk")
                emit_fc(1)
                ln_apply(yy, cB0, 2, 3, 0, final_out=outT)
                emit_out(0)
                sB1 = ln_stats(yy, 1, "b", "proj")
                cB1 = ln_chain(*sB1, 1, "b", "proj")
                ln_apply(yy, cB1, 2, 3, 1, final_out=outT)
                emit_out(1)


def _build(NT):
    key = ("nc", NT)
    if key in _CACHE:
        return _CACHE[key]
    nc = bacc.Bacc(None, target_bir_lowering=False, debug=False)
    with tile.TileContext(nc) as tc:
        _emit(nc, tc, NT)
    nc.compile()
    _CACHE[key] = nc
    return nc


def _ptile(a):
    # [n, m] -> transpose -> [m(=tiles*128), n] -> [128, tiles, n]
    t = np.asarray(a, np.float32).T
    return np.ascontiguousarray(
        t.reshape(DT, P, t.shape[1]).transpose(1, 0, 2))


def _mask_nt(mask):
    mask = np.asarray(mask)
    counts = mask.sum(axis=1)
    return max(1, int(np.ceil(counts.max() / P)))


def _prep_in_maps(q, k, v, mask, Wq, Wk, Wv, fc_w, fc_b, g0, b0, g1, b1):
    q = np.asarray(q, np.float32)
    k = np.asarray(k, np.float32)
    v = np.asarray(v, np.float32)
    mask = np.asarray(mask)
    bf = mybir.dt.np(BF16)
    f8 = mybir.dt.np(FP8)
    NT = _mask_nt(mask)
    NPAD = NT * P

    WqTh = _ptile(Wq).astype(bf)
    WkTh = _ptile(Wk).astype(bf)
    Wv8h = _ptile(np.clip(np.asarray(Wv, np.float32), -240, 240)).astype(f8)
    fcwTh = _ptile(fc_w).astype(bf)
    vecs = np.stack([np.asarray(x, np.float32).reshape(DT, P).T
                     for x in (g0, b0, g1, b1, fc_b)])
    vecs = np.ascontiguousarray(vecs)

    in_maps = []
    for c in range(NCORES):
        b = c // 2
        r0 = (c % 2) * LQ
        qTb = _ptile(q[b][r0:r0 + LQ]).astype(bf)
        qrTb = qTb.copy()
        kTh = np.zeros((H, P, DT, NPAD), bf)
        vT8h = np.zeros((H, P, DT, NPAD), f8)
        mbh = np.full((P, H, NT), np.float32(NEG), np.float32)
        for h in range(H):
            ii = np.nonzero(mask[h * B + b])[0]
            n = len(ii)
            kp = np.zeros((NPAD, D), np.float32)
            kp[:n] = k[b][ii]
            vp = np.zeros((NPAD, D), np.float32)
            vp[:n] = np.clip(v[b][ii], -240, 240)
            kTh[h] = _ptile(kp).astype(bf)
            vT8h[h] = _ptile(vp).astype(f8)
            bias = np.full(NPAD, np.float32(NEG), np.float32)
            bias[:n] = 0.0
            mbh[:, h, :] = bias.reshape(NT, P).T
        in_maps.append({
            "qT": qTb, "kT": kTh, "vT8": vT8h, "qresT": qrTb,
            "WqT": WqTh, "WkT": WkTh, "WvT8": Wv8h, "fcwT": fcwTh,
            "mb": np.ascontiguousarray(mbh.reshape(P, H * NT)),
            "vecs": vecs,
        })
    return in_maps, NT


def kernel(q, k, v, mask, Wq, Wk, Wv, fc_w, fc_b, g0, b0, g1, b1):
    in_maps, NT = _prep_in_maps(q, k, v, mask, Wq, Wk, Wv, fc_w, fc_b,
                                g0, b0, g1, b1)
    nc = _build(NT)
    res = run_bass_kernel_spmd(nc, in_maps, core_ids=list(range(NCORES)))
    outf = np.empty((B, L, D), np.float32)
    for c in range(NCORES):
        b = c // 2
        r0 = (c % 2) * LQ
        o = np.asarray(res.results[c]["out"], np.float32)  # [128, DT, LQ]
        outf[b, r0:r0 + LQ, :] = o.transpose(2, 1, 0).reshape(LQ, D)
    return outf
